# revision 15
# baseline (speedup 1.0000x reference)
"""Causal self-attention with ALiBi on 8 TRN2 NeuronCores.

Sharding: 8 cores = 4 batches x 2 head-groups (8 heads each).
Each core computes q/k/v projections for its (batch, head-group) shard,
causal attention in "ST layout" (scores kept k-major so the P@V matmul
needs no transposes), and a partial output projection.  Host sums the two
head-group partials per batch and adds bo.

All matmuls run as float32r (TF32-class, full PE rate at N>=256).
ALiBi is applied via one DVE op per score chunk:
    U = S_raw * c_h + REL,   c_h = 1/(sqrt(d)*slope_h)
    P = exp(U * slope_h + slope_h*128*j)     (ACT, scale/bias per-partition APs)
which equals exp(qk/sqrt(d) + slope_h*(k_pos - q_pos)).  The causal mask
lives in the REL_DIAG tile (-3e38 above the diagonal).  The softmax
denominator comes from a ones-column appended to v (stationary operand of
the P@V matmul); normalization is done on the d-major attention output via
a one-hot broadcast matmul of the reciprocal denominators.
"""

import math
from contextlib import ExitStack

import numpy as np

import concourse.bass as bass
import concourse.mybir as mybir
import concourse.tile as tile
from concourse import bacc
from concourse.masks import make_identity

F32 = mybir.dt.float32
F32R = mybir.dt.float32r
AF = mybir.ActivationFunctionType
ALU = mybir.AluOpType

P = 128
D_MODEL = 1024
N_HEADS = 16
HEAD_DIM = 64
B = 4
L_FULL = 2048
HPC = 8               # heads per core
DPC = HPC * HEAD_DIM  # 512 features per core
N_CORES = 8


def _alibi_slopes(n_heads):
    def pow2_slopes(n):
        start = 2.0 ** (-(2.0 ** (-(math.log2(n) - 3))))
        return [start * (start ** i) for i in range(n)]
    if math.log2(n_heads).is_integer():
        s = pow2_slopes(n_heads)
    else:
        closest = 2 ** math.floor(math.log2(n_heads))
        s = pow2_slopes(closest)
        extra = pow2_slopes(2 * closest)
        s = s + extra[0::2][: n_heads - closest]
    return np.array(s, dtype=np.float32)


def _chunks(q0, L):
    """Bank-aligned (start, width) chunks covering [q0, L)."""
    out = []
    cs = q0
    while cs < L:
        ce = min(L, (cs // 512 + 1) * 512)
        out.append((cs, ce - cs))
        cs = ce
    return out


def build_nc(L=L_FULL):
    NLT = L // P        # L tiles
    NKT = D_MODEL // P  # input-feature K tiles
    NMT = DPC // P      # output-feature M tiles (4)
    NC512 = L // 512    # 512-wide chunks across L

    nc = bacc.Bacc(None, target_bir_lowering=False, debug=False)

    xT_d = nc.declare_dram_parameter("xT", [D_MODEL, L], F32, isOutput=False)
    wqT_d = nc.declare_dram_parameter("wqT", [D_MODEL, DPC], F32, isOutput=False)
    wkT_d = nc.declare_dram_parameter("wkT", [D_MODEL, DPC], F32, isOutput=False)
    wvT_d = nc.declare_dram_parameter("wvT", [D_MODEL, DPC], F32, isOutput=False)
    woT_d = nc.declare_dram_parameter("woT", [DPC, D_MODEL], F32, isOutput=False)
    bqT_d = nc.declare_dram_parameter("bqT", [P, NMT], F32, isOutput=False)
    bkT_d = nc.declare_dram_parameter("bkT", [P, NMT], F32, isOutput=False)
    bvb_d = nc.declare_dram_parameter("bvb", [P, DPC], F32, isOutput=False)
    rel_d = nc.declare_dram_parameter("rel", [P, L], F32, isOutput=False)
    reld_d = nc.declare_dram_parameter("reld", [P, P], F32, isOutput=False)
    acon_d = nc.declare_dram_parameter("acon", [P, 144], F32, isOutput=False)
    eyeb_d = nc.declare_dram_parameter("eyeb", [8, 1024], F32, isOutput=False)
    delt_d = nc.declare_dram_parameter("delt", [64, P], F32, isOutput=False)
    out_d = nc.declare_dram_parameter("out_p", [L, D_MODEL], F32, isOutput=True)
    k_d = nc.declare_dram_parameter("k_out", [HPC, L, HEAD_DIM], F32, isOutput=True)
    v_d = nc.declare_dram_parameter("v_out", [HPC, L, HEAD_DIM], F32, isOutput=True)

    with tile.TileContext(nc) as tc, ExitStack() as top:
        const = top.enter_context(tc.tile_pool(name="const", bufs=1))
        rel = const.tile([P, L], F32)
        nc.sync.dma_start(out=rel[:, :], in_=rel_d[:, :])
        reld = const.tile([P, P], F32)
        nc.sync.dma_start(out=reld[:, :], in_=reld_d[:, :])
        acon = const.tile([P, 144], F32)
        nc.sync.dma_start(out=acon[:, :], in_=acon_d[:, :])
        bqT = const.tile([P, NMT], F32)
        nc.sync.dma_start(out=bqT[:, :], in_=bqT_d[:, :])
        bkT = const.tile([P, NMT], F32)
        nc.sync.dma_start(out=bkT[:, :], in_=bkT_d[:, :])
        bvb = const.tile([P, DPC], F32)
        nc.sync.dma_start(out=bvb[:, :], in_=bvb_d[:, :])
        eyeb0 = const.tile([8, 1024], F32)
        nc.sync.dma_start(out=eyeb0[:, :], in_=eyeb_d[:, :])
        eyeb = const.tile([8, 1024], F32R)
        nc.vector.tensor_copy(eyeb[:, :], eyeb0[:, :])
        delt0 = const.tile([64, P], F32)
        nc.sync.dma_start(out=delt0[:, :], in_=delt_d[:, :])
        delt = const.tile([64, P], F32R)
        nc.vector.tensor_copy(delt[:, :], delt0[:, :])
        ident = const.tile([P, P], F32)
        make_identity(nc, ident[:, :])

        # persistent activations (live from projections through o-proj)
        persist = top.enter_context(tc.tile_pool(name="persist", bufs=1))
        qTr = [persist.tile([P, L], F32R, name=f"qTr{m}") for m in range(NMT)]
        kTr = [persist.tile([P, L], F32R, name=f"kTr{m}") for m in range(NMT)]
        # v with ones column per head: even head h -> [v(64) | 1] at cols 65h,
        # odd head h -> [1 | v(64)].
        vS = [persist.tile([P, 65 * HPC], F32R, name=f"vS{i}") for i in range(NLT)]

        # ---------------- q/k projections (stream xT per 512-col chunk) -----
        with ExitStack() as qk:
            wpool = qk.enter_context(tc.tile_pool(name="wqk", bufs=1))
            wstage = qk.enter_context(tc.tile_pool(name="wstage", bufs=2))
            xs = qk.enter_context(tc.tile_pool(name="xs", bufs=NKT + 2))
            xstage = qk.enter_context(tc.tile_pool(name="xstage", bufs=2))
            psp = qk.enter_context(tc.tile_pool(name="psp", bufs=3, space="PSUM"))
            wqr, wkr = [], []
            for kt in range(NKT):
                for (wd, lst, wtag) in ((wqT_d, wqr, "wq"), (wkT_d, wkr, "wk")):
                    ws = wstage.tile([P, DPC], F32, tag="wstage")
                    nc.sync.dma_start(out=ws[:, :], in_=wd[kt * P:(kt + 1) * P, :])
                    wr = wpool.tile([P, DPC], F32R, tag=f"{wtag}{kt}")
                    nc.vector.tensor_copy(wr[:, :], ws[:, :])
                    lst.append(wr)
            for n in range(NC512):
                xsr = []
                for kt in range(NKT):
                    st = xstage.tile([P, 512], F32, tag="xstage")
                    nc.sync.dma_start(
                        out=st[:, :],
                        in_=xT_d[kt * P:(kt + 1) * P, n * 512:(n + 1) * 512])
                    xr = xs.tile([P, 512], F32R, tag="xr")
                    nc.vector.tensor_copy(xr[:, :], st[:, :])
                    xsr.append(xr)
                for (wr, bias, dst) in ((wqr, bqT, qTr), (wkr, bkT, kTr)):
                    for m in range(NMT):
                        ps = psp.tile([P, 512], F32, tag="psp")
                        for kt in range(NKT):
                            nc.tensor.matmul(
                                ps[:, :],
                                wr[kt][:, m * P:(m + 1) * P],
                                xsr[kt][:, :],
                                start=(kt == 0), stop=(kt == NKT - 1),
                            )
                        nc.vector.tensor_scalar_add(
                            dst[m][:, n * 512:(n + 1) * 512], ps[:, :],
                            bias[:, m:m + 1])

        # ---------------- k_out: transpose kT to natural layout ----------
        with ExitStack() as ktr:
            pst = ktr.enter_context(tc.tile_pool(name="pst", bufs=2, space="PSUM"))
            kevac = ktr.enter_context(tc.tile_pool(name="kevac", bufs=3))
            for m in range(NMT):
                for i in range(NLT):
                    pt = pst.tile([P, P], F32, tag="pst")
                    nc.tensor.transpose(
                        pt[:, :], kTr[m][:, i * P:(i + 1) * P].bitcast(F32),
                        ident[:, :])
                    ke = kevac.tile([P, P], F32, tag="kevac")
                    nc.vector.tensor_copy(ke[:, :], pt[:, :])
                    for hh in range(2):
                        nc.sync.dma_start(
                            out=k_d[2 * m + hh, i * P:(i + 1) * P, :],
                            in_=ke[:, hh * 64:(hh + 1) * 64])

        # ---------------- v projection (natural layout) ----------------
        with ExitStack() as vp:
            wvpool = vp.enter_context(tc.tile_pool(name="wv", bufs=NKT))
            wstage2 = vp.enter_context(tc.tile_pool(name="wstage2", bufs=2))
            xv = vp.enter_context(tc.tile_pool(name="xv", bufs=NKT + 2))
            xvstage = vp.enter_context(tc.tile_pool(name="xvstage", bufs=2))
            psv = vp.enter_context(tc.tile_pool(name="psv", bufs=2, space="PSUM"))
            wvr = []
            for kt in range(NKT):
                ws = wstage2.tile([P, DPC], F32, tag="wstage2")
                nc.sync.dma_start(out=ws[:, :], in_=wvT_d[kt * P:(kt + 1) * P, :])
                wr = wvpool.tile([P, DPC], F32R, tag="wv")
                nc.vector.tensor_copy(wr[:, :], ws[:, :])
                wvr.append(wr)
            for i in range(NLT):
                xvr = []
                for kt in range(NKT):
                    st = xvstage.tile([P, P], F32, tag="xvstage")
                    nc.sync.dma_start(
                        out=st[:, :],
                        in_=xT_d[kt * P:(kt + 1) * P, i * P:(i + 1) * P])
                    xr = xv.tile([P, P], F32R, tag="xvr")
                    nc.vector.tensor_copy(xr[:, :], st[:, :])
                    xvr.append(xr)
                ps = psv.tile([P, DPC], F32, tag="psv")
                for kt in range(NKT):
                    nc.tensor.matmul(
                        ps[:, :], xvr[kt][:, :], wvr[kt][:, :],
                        start=(kt == 0), stop=(kt == NKT - 1),
                    )
                vt = vS[i]
                vr = vt.rearrange("p (h c) -> p h c", c=65)
                pr = ps.rearrange("p (h c) -> p h c", c=64)
                br = bvb.rearrange("p (h c) -> p h c", c=64)
                # all heads: [v(64) | 1] at cols 65h.  Ones column via
                # tensor_scalar (in0*0 + 1) -- memset can't write f32r.
                nc.vector.scalar_tensor_tensor(
                    out=vr[:, :, 0:64], in0=pr[:, :, :], scalar=1.0,
                    in1=br[:, :, :], op0=ALU.mult, op1=ALU.add)
                nc.vector.tensor_scalar(
                    out=vr[:, :, 64:65],
                    in0=bvb[:, 0:8].rearrange("p (h o) -> p h o", o=1),
                    scalar1=0.0, scalar2=1.0, op0=ALU.mult, op1=ALU.add)
                for h in range(HPC):
                    nc.sync.dma_start(
                        out=v_d[h, i * P:(i + 1) * P, :],
                        in_=vt[:, h * 65:h * 65 + 64].bitcast(F32))

        # ---------------- attention + o-proj ----------------
        with ExitStack() as big:
            opool = big.enter_context(tc.tile_pool(name="opool", bufs=1))
            oTn = [opool.tile([P, L], F32R, name=f"oTn{m}") for m in range(NMT)]
            with ExitStack() as att:
                ps_s = att.enter_context(
                    tc.tile_pool(name="ps_s", bufs=2, space="PSUM"))
                ps_oT = att.enter_context(
                    tc.tile_pool(name="ps_oT", bufs=1, space="PSUM"))
                ps_den = att.enter_context(
                    tc.tile_pool(name="ps_den", bufs=2, space="PSUM"))
                upool = att.enter_context(tc.tile_pool(name="upool", bufs=2))
                ppool = att.enter_context(tc.tile_pool(name="ppool", bufs=2))
                dpool = att.enter_context(tc.tile_pool(name="dpool", bufs=1))
                dnpool = att.enter_context(tc.tile_pool(name="dn8", bufs=1))
                den8 = dnpool.tile([8, L], F32)
                rec8 = dnpool.tile([8, L], F32R)
                nc.vector.memset(den8[:, :], 1.0)
                nc.vector.tensor_copy(rec8[:, :], den8[:, :])

                jmax = {b: min(NLT - 1, (512 * b + 511) // P)
                        for b in range(NC512)}
                for h in range(HPC):
                    mh, hb = h // 2, 64 * (h % 2)
                    par = h % 2
                    oT = ps_oT.tile([P, L], F32, tag="oT")
                    for j in range(NLT):
                        q0 = P * j
                        for (cs, cw) in _chunks(q0, L):
                            sp = ps_s.tile([P, 512], F32, tag="sp")
                            nc.tensor.matmul(
                                sp[:, 0:cw],
                                kTr[mh][hb:hb + 64, j * P:(j + 1) * P],
                                qTr[mh][hb:hb + 64, cs:cs + cw],
                                start=True, stop=True,
                            )
                            ut = upool.tile([P, 512], F32, tag="ut")
                            nc.vector.scalar_tensor_tensor(
                                out=ut[:, 0:cw], in0=sp[:, 0:cw],
                                scalar=acon[:, h:h + 1],
                                in1=rel[:, cs:cs + cw],
                                op0=ALU.mult, op1=ALU.add)
                            if cs == q0:  # chunk contains the diagonal block
                                nc.vector.tensor_tensor(
                                    out=ut[:, 0:P], in0=ut[:, 0:P],
                                    in1=reld[:, :], op=ALU.add)
                            pt = ppool.tile([P, 512], F32R, tag="pt")
                            b_col = 16 + h * 16 + j
                            nc.scalar.activation(
                                pt[:, 0:cw], ut[:, 0:cw], AF.Exp,
                                bias=acon[:, b_col:b_col + 1],
                                scale=acon[:, 8 + h:9 + h])
                            bank = cs // 512
                            nc.tensor.matmul(
                                oT[0:65, cs:cs + cw],
                                vS[j][:, 65 * h:65 * h + 65],
                                pt[:, 0:cw],
                                start=(j == 0), stop=(j == jmax[bank]),
                            )
                    # denominator -> den8 row h (sbuf bounce + dma)
                    drow = dpool.tile([P, L], F32, tag="drow")
                    nc.scalar.copy(drow[64:65, :], oT[64:65, :])
                    nc.sync.dma_start(out=den8[h:h + 1, :],
                                      in_=drow[64:65, :])
                    # evac unnormalized o; odd heads move rows 0-63 -> 64-127
                    if par == 0:
                        nc.vector.tensor_copy(oTn[mh][0:64, :], oT[0:64, :])
                    else:
                        for b in range(NC512):
                            tmp = upool.tile([P, 512], F32R, tag="otmp")
                            nc.vector.tensor_copy(
                                tmp[0:64, :],
                                oT[0:64, b * 512:(b + 1) * 512])
                            mv = ps_den.tile([P, 512], F32, tag="dn")
                            nc.tensor.matmul(
                                mv[:, :], delt[:, :], tmp[0:64, :],
                                start=True, stop=True,
                            )
                            nc.vector.tensor_copy(
                                oTn[mh][64:128, b * 512:(b + 1) * 512],
                                mv[64:128, :])

                # batched reciprocal + normalize all heads in place
                with nc.allow_low_precision(reason="f32r recip, tf32 ok"):
                    nc.vector.reciprocal(rec8[:, :], den8[:, :])
                for h in range(HPC):
                    mh, hb = h // 2, 64 * (h % 2)
                    for b in range(NC512):
                        dn = ps_den.tile([P, 512], F32, tag="dn")
                        nc.tensor.matmul(
                            dn[:, :],
                            eyeb[:, h * P:(h + 1) * P],
                            rec8[:, b * 512:(b + 1) * 512],
                            start=True, stop=True,
                        )
                        nc.vector.tensor_tensor(
                            out=oTn[mh][hb:hb + 64, b * 512:(b + 1) * 512],
                            in0=oTn[mh][hb:hb + 64, b * 512:(b + 1) * 512],
                            in1=dn[hb:hb + 64, :],
                            op=ALU.mult)

            # ---------------- output projection ----------------
            with ExitStack() as op:
                wopool = op.enter_context(tc.tile_pool(name="wo", bufs=NMT))
                wstage3 = op.enter_context(tc.tile_pool(name="wstage3", bufs=2))
                pso = op.enter_context(
                    tc.tile_pool(name="pso", bufs=3, space="PSUM"))
                oevac = op.enter_context(tc.tile_pool(name="oevac", bufs=3))
                wor = []
                for m in range(NMT):
                    ws = wstage3.tile([P, D_MODEL], F32, tag="wstage3")
                    nc.sync.dma_start(out=ws[:, :],
                                      in_=woT_d[m * P:(m + 1) * P, :])
                    wr = wopool.tile([P, D_MODEL], F32R, tag="wo")
                    nc.vector.tensor_copy(wr[:, :], ws[:, :])
                    wor.append(wr)
                for i in range(NLT):
                    for ofc in range(2):
                        ps = pso.tile([P, 512], F32, tag="pso")
                        for m in range(NMT):
                            nc.tensor.matmul(
                                ps[:, :],
                                oTn[m][:, i * P:(i + 1) * P],
                                wor[m][:, ofc * 512:(ofc + 1) * 512],
                                start=(m == 0), stop=(m == NMT - 1),
                            )
                        oe = oevac.tile([P, 512], F32, tag="oevac")
                        nc.vector.tensor_copy(oe[:, :], ps[:, :])
                        nc.sync.dma_start(
                            out=out_d[i * P:(i + 1) * P,
                                      ofc * 512:(ofc + 1) * 512],
                            in_=oe[:, :])

    nc.finalize()
    return nc


# ---------------- host side ----------------

def make_core_inputs(inputs, core, L=L_FULL):
    x = np.asarray(inputs["x"], dtype=np.float32)
    b, hg = core // 2, core % 2
    hsl = slice(hg * DPC, (hg + 1) * DPC)
    heads = range(hg * HPC, hg * HPC + HPC)
    slopes = _alibi_slopes(N_HEADS)
    sq = math.sqrt(HEAD_DIM)
    NMT = DPC // P

    qq = np.arange(L, dtype=np.float32)
    kk = np.arange(P, dtype=np.float32)
    rel = kk[:, None] - qq[None, :]
    m = kk[:, None] - kk[None, :]
    reld = np.where(m > 0, np.float32(-3e38), np.float32(0.0)).astype(np.float32)

    acon = np.zeros((P, 144), dtype=np.float32)
    for i, h in enumerate(heads):
        s = slopes[h]
        acon[:, i] = 1.0 / (sq * s)
        acon[:, 8 + i] = s
        for j in range(L // P):
            acon[:, 16 + i * 16 + j] = s * P * j

    eyeb = np.zeros((8, 1024), dtype=np.float32)
    for i in range(8):
        eyeb[i, i * 128:(i + 1) * 128] = 1.0
    delt = np.zeros((64, 128), dtype=np.float32)
    for r in range(64):
        delt[r, 64 + r] = 1.0

    return {
        "xT": np.ascontiguousarray(x[b].T),
        "wqT": np.ascontiguousarray(np.asarray(inputs["Wq"])[hsl, :].T),
        "wkT": np.ascontiguousarray(np.asarray(inputs["Wk"])[hsl, :].T),
        "wvT": np.ascontiguousarray(np.asarray(inputs["Wv"])[hsl, :].T),
        "woT": np.ascontiguousarray(np.asarray(inputs["Wo"])[:, hsl].T),
        "bqT": np.ascontiguousarray(
            np.asarray(inputs["bq"], dtype=np.float32)[hsl].reshape(NMT, P).T),
        "bkT": np.ascontiguousarray(
            np.asarray(inputs["bk"], dtype=np.float32)[hsl].reshape(NMT, P).T),
        "bvb": np.broadcast_to(
            np.asarray(inputs["bv"], dtype=np.float32)[hsl], (P, DPC)).copy(),
        "rel": np.ascontiguousarray(rel, dtype=np.float32),
        "reld": reld,
        "acon": acon,
        "eyeb": eyeb,
        "delt": delt,
    }


_NC_CACHE = {}


def kernel(**inputs):
    from concourse.bass_utils import run_bass_kernel_spmd

    L = L_FULL
    if L not in _NC_CACHE:
        _NC_CACHE[L] = build_nc(L)
    nc = _NC_CACHE[L]
    in_maps = [make_core_inputs(inputs, c, L) for c in range(N_CORES)]
    res = run_bass_kernel_spmd(nc, in_maps, list(range(N_CORES))).results

    bo = np.asarray(inputs["bo"], dtype=np.float32)
    out = np.stack([
        res[2 * b]["out_p"] + res[2 * b + 1]["out_p"] + bo for b in range(B)
    ])
    k = np.stack([
        np.concatenate([res[2 * b]["k_out"], res[2 * b + 1]["k_out"]], axis=0)
        for b in range(B)
    ])
    v = np.stack([
        np.concatenate([res[2 * b]["v_out"], res[2 * b + 1]["v_out"]], axis=0)
        for b in range(B)
    ])
    return out, k, v
